# revision 3
# baseline (speedup 1.0000x reference)
"""GCN + MLP concat kernel for Trainium2, 8-core SPMD.

Model (reference.py):
    gcn_out = relu(gcn_conv(xfeat, edge_index, W_gcn, b_gcn))      # symmetric-norm GCN
    mlp_out = relu(concat(xfeat, xlabel) @ W_mlp + b_mlp)
    out     = concat(gcn_out, mlp_out) @ W_cls + b_cls

Shapes: N=100000 nodes, E=1600000 edges, XF=128, XL=40, H=128, C=40.

Strategy (sharding_hint): shard dst nodes across 8 cores (12500 each,
padded to 12800 = 100 blocks of 128); each core handles edges into its
shard; weights replicated.

v2 design (norm factorization):
    norm_e = dinv[src]*dinv[dst]  =>  gather xs = dinv (.) xfeat (bf16),
    aggregate with 0/1 selection matmuls (S streamed as fp8: exact 1.0),
    self-loop via identity matmul on local shard rows, and the dinv[dst]
    factor applied exactly in fp32 by the ACT-engine PSUM evacuation
    (per-partition scale).  relu(dinv*z @ W) is exact since dinv >= 0.
    MLP inputs are pre-transposed host-side (no PE transposes for them).
    Gather: 40 dma_gather calls of 6400 rows (vs 80x3200) to amortize
    SWDGE fixed overhead; idx streamed per gather-group to save SBUF.
"""

import numpy as np
import ml_dtypes

N, E = 100000, 1600000
XF, XL, H, C = 128, 40, 128, 40
NCORES = 8
NSHARD = N // NCORES          # 12500 dst nodes per core
P = 128
NBLK = 100                    # dst blocks per core (12800 padded rows)
NPAD = NBLK * P               # 12800
NQ = 4                        # src-table quartiles (int16 index range)
QROWS = N // NQ               # 25000
TBQ = 5                       # gather tiles per (block, quartile) - 640 slots
SB_BLKS = 10                  # blocks per gather group
NSB = NBLK // SB_BLKS         # 10 gather groups
TSB = NQ * SB_BLKS * TBQ      # 200 tiles per gather group
TTOT = NBLK * NQ * TBQ        # 2000 tiles per core
CALL = SB_BLKS * TBQ * P      # 6400 slots per gather call (per quartile)

BF16 = ml_dtypes.bfloat16
FP8 = ml_dtypes.float8_e4m3fn


def _preprocess(xfeat, xlabel, edge_index, dinv):
    """Host-side sharding/layout. Returns per-core input dicts' arrays."""
    src = np.ascontiguousarray(edge_index[0]).astype(np.int64)
    dst = np.ascontiguousarray(edge_index[1]).astype(np.int64)

    core = dst // NSHARD
    blk = (dst % NSHARD) // P
    qrt = src // QROWS
    dloc = (dst % NSHARD) % P  # position within block

    # order edges by (core, block, quartile, src)
    order = np.lexsort((src, qrt, blk, core))
    src_s = src[order]
    core_s = core[order]
    blk_s = blk[order]
    qrt_s = qrt[order]
    dloc_s = dloc[order]

    cell = ((core_s * NBLK + blk_s) * NQ + qrt_s)  # global (c,b,q) cell id
    ncells = NCORES * NBLK * NQ
    counts = np.bincount(cell, minlength=ncells)
    if counts.max() > TBQ * P:
        raise RuntimeError(f"cell overflow: {counts.max()} > {TBQ * P}")
    cell_starts = np.zeros(ncells, np.int64)
    cell_starts[1:] = np.cumsum(counts)[:-1]
    within = np.arange(len(src_s)) - cell_starts[cell]

    # global slot id per edge; slot layout per core:
    # for g in NSB: for q in NQ: for b in SB_BLKS: TBQ tiles of 128 slots
    b_, q_ = blk_s, qrt_s
    tile_base = ((b_ // SB_BLKS) * TSB + q_ * (SB_BLKS * TBQ)
                 + (b_ % SB_BLKS) * TBQ)
    slot = tile_base * P + within
    gslot = core_s * (TTOT * P) + slot

    total_slots = NCORES * TTOT * P
    idx_flat = np.zeros(total_slots, np.int16)
    dloc_flat = np.zeros(total_slots, np.int64)
    val_flat = np.zeros(total_slots, np.float32)
    idx_flat[gslot] = (src_s - q_ * QROWS).astype(np.int16)
    dloc_flat[gslot] = dloc_s
    val_flat[gslot] = 1.0

    cores = []
    for c in range(NCORES):
        s0, s1 = c * TTOT * P, (c + 1) * TTOT * P
        idx_c = idx_flat[s0:s1]
        # idx wrap for dma_gather: per call region, idx j at [j%16, j//16],
        # replicated to the 8 16-partition groups.
        ncalls = TTOT * P // CALL
        w = idx_c.reshape(ncalls, CALL // 16, 16)          # [call, col, 16]
        w = np.transpose(w, (2, 0, 1)).reshape(16, TTOT * P // 16)
        idx_wrapped = np.tile(w, (8, 1))

        # host-built 0/1 selection tiles S^T (fp8): [128 slots, TTOT, 128 dst]
        sarr = np.zeros((P, TTOT, P), FP8)
        pp = (np.arange(TTOT * P) % P)
        tt = (np.arange(TTOT * P) // P)
        sarr[pp, tt, dloc_flat[s0:s1]] = val_flat[s0:s1].astype(FP8)
        sarr = sarr.reshape(P, TTOT * P)

        nodes0 = c * NSHARD
        xf_shard = np.zeros((NPAD, XF), np.float32)
        xf_shard[:NSHARD] = xfeat[nodes0:nodes0 + NSHARD]
        xl_shard = np.zeros((NPAD, XL), np.float32)
        xl_shard[:NSHARD] = xlabel[nodes0:nodes0 + NSHARD]
        d_pad = np.zeros(NPAD, np.float32)
        d_pad[:NSHARD] = dinv[nodes0:nodes0 + NSHARD]
        dinvc = d_pad.reshape(NBLK, P).T.copy()            # [128, NBLK]
        # local shard of the scaled table for the self-loop identity matmul
        xss = (d_pad[:, None] * xf_shard).astype(BF16)     # [NPAD, XF]

        cores.append(dict(
            idx=idx_wrapped, sarr=sarr, xss=xss,
            xfT=np.ascontiguousarray(xf_shard.T).astype(BF16),
            xlT=np.ascontiguousarray(xl_shard.T).astype(BF16),
            dinvc=dinvc,
        ))
    return cores


def _build_bass():
    import concourse.mybir as mybir
    import concourse.tile as tile
    from concourse import bacc
    from concourse.masks import make_identity

    f32 = mybir.dt.float32
    bf16 = mybir.dt.bfloat16
    f8 = mybir.dt.float8e4
    i16 = mybir.dt.int16
    AF = mybir.ActivationFunctionType

    nc = bacc.Bacc(None, target_bir_lowering=False, num_swdge_queues=4)

    xsbf = nc.dram_tensor("xsbf", [N, XF], bf16, kind="ExternalInput")
    idx = nc.dram_tensor("idx", [P, TTOT * P // 16], i16, kind="ExternalInput")
    sarr = nc.dram_tensor("sarr", [P, TTOT * P], f8, kind="ExternalInput")
    xss = nc.dram_tensor("xss", [NPAD, XF], bf16, kind="ExternalInput")
    xfT = nc.dram_tensor("xfT", [XF, NPAD], bf16, kind="ExternalInput")
    xlT = nc.dram_tensor("xlT", [XL, NPAD], bf16, kind="ExternalInput")
    dinvc = nc.dram_tensor("dinvc", [P, NBLK], f32, kind="ExternalInput")
    wgcn = nc.dram_tensor("wgcn", [XF, H], f32, kind="ExternalInput")
    wmlpf = nc.dram_tensor("wmlpf", [XF, H], bf16, kind="ExternalInput")
    wmlpl = nc.dram_tensor("wmlpl", [XL, H], bf16, kind="ExternalInput")
    wclsg = nc.dram_tensor("wclsg", [H, C], f32, kind="ExternalInput")
    wclsm = nc.dram_tensor("wclsm", [H, C], f32, kind="ExternalInput")
    bmlp = nc.dram_tensor("bmlp", [H, 1], f32, kind="ExternalInput")
    bcls = nc.dram_tensor("bcls", [C, 1], f32, kind="ExternalInput")

    out = nc.dram_tensor("out", [NPAD, C], f32, kind="ExternalOutput")

    ICALL = CALL * NQ // 16  # idx columns per gather group (1600)

    with tile.TileContext(nc) as tc:
        with (
            tc.tile_pool(name="const", bufs=1) as cpool,
            tc.tile_pool(name="meta", bufs=2) as mpool,
            tc.tile_pool(name="gbuf", bufs=2) as gpool,
            tc.tile_pool(name="sbufS", bufs=2) as spool,
            tc.tile_pool(name="work", bufs=3) as wpool,
            tc.tile_pool(name="head", bufs=3) as hpool,
            tc.tile_pool(name="psA", bufs=2, space="PSUM") as psA,
            tc.tile_pool(name="psB", bufs=2, space="PSUM") as psB,
            tc.tile_pool(name="psC", bufs=1, space="PSUM") as psC,
        ):
            ident = cpool.tile([P, P], f32)
            make_identity(nc, ident[:])
            ident_bf = cpool.tile([P, P], bf16)
            make_identity(nc, ident_bf[:])
            wgcn_t = cpool.tile([XF, H], f32)
            nc.sync.dma_start(out=wgcn_t[:], in_=wgcn[:, :])
            wmlpf_t = cpool.tile([XF, H], bf16)
            nc.sync.dma_start(out=wmlpf_t[:], in_=wmlpf[:, :])
            wmlpl_t = cpool.tile([XL, H], bf16)
            nc.sync.dma_start(out=wmlpl_t[:], in_=wmlpl[:, :])
            wclsg_t = cpool.tile([H, C], f32)
            nc.sync.dma_start(out=wclsg_t[:], in_=wclsg[:, :])
            wclsm_t = cpool.tile([H, C], f32)
            nc.sync.dma_start(out=wclsm_t[:], in_=wclsm[:, :])
            bmlp_t = cpool.tile([H, 1], f32)
            nc.sync.dma_start(out=bmlp_t[:], in_=bmlp[:, :])
            bcls_t = cpool.tile([C, 1], f32)
            nc.sync.dma_start(out=bcls_t[:], in_=bcls[:, :])
            dinvc_t = cpool.tile([P, NBLK], f32)
            nc.sync.dma_start(out=dinvc_t[:], in_=dinvc[:, :])

            for g in range(NSB):
                idx_t = mpool.tile([P, ICALL], i16, tag="idx")
                nc.sync.dma_start(
                    out=idx_t[:], in_=idx[:, g * ICALL:(g + 1) * ICALL])
                g_t = gpool.tile([P, TSB, P], bf16, tag="g")
                for q in range(NQ):
                    nc.gpsimd.dma_gather(
                        g_t[:, q * SB_BLKS * TBQ:(q + 1) * SB_BLKS * TBQ, :],
                        xsbf[q * QROWS:(q + 1) * QROWS, :],
                        idx_t[:, q * (CALL // 16):(q + 1) * (CALL // 16)],
                        CALL, CALL, P,
                        single_packet=False,
                        queue_num=(g * NQ + q) % 4,
                    )
                s_t = spool.tile([P, TSB * P], f8, tag="sm")
                nc.sync.dma_start(
                    out=s_t[:], in_=sarr[:, g * TSB * P:(g + 1) * TSB * P])
                for bl in range(SB_BLKS):
                    b = g * SB_BLKS + bl
                    z_ps = psA.tile([P, P], f32, tag="z")
                    for q in range(NQ):
                        for k in range(TBQ):
                            t_in_sb = q * (SB_BLKS * TBQ) + bl * TBQ + k
                            nc.tensor.matmul(
                                out=z_ps[:],
                                lhsT=s_t[:, t_in_sb * P:(t_in_sb + 1) * P],
                                rhs=g_t[:, t_in_sb, :],
                                start=(q == 0 and k == 0),
                                stop=False,
                            )
                    # self-loop: z += I.T @ xs_local  (scaled table rows)
                    xss_t = wpool.tile([P, XF], bf16, tag="xss")
                    nc.sync.dma_start(out=xss_t[:], in_=xss[b * P:(b + 1) * P, :])
                    nc.tensor.matmul(
                        out=z_ps[:], lhsT=ident_bf[:], rhs=xss_t[:],
                        start=False, stop=True,
                    )
                    # PSUM evacuation with exact dinv[dst] scaling (ACT engine)
                    z_sb = wpool.tile([P, P], f32, tag="zsb")
                    nc.scalar.activation(out=z_sb[:], in_=z_ps[:], func=AF.Copy,
                                         scale=dinvc_t[:, b:b + 1])
                    zT_ps = psB.tile([P, P], f32, tag="tp")
                    nc.tensor.transpose(out=zT_ps[:], in_=z_sb[:], identity=ident[:])
                    zT = wpool.tile([P, P], f32, tag="zTs")
                    nc.scalar.activation(out=zT[:], in_=zT_ps[:], func=AF.Copy)
                    # heads (feature-major)
                    gcn_ps = psC.tile([H, P], f32, tag="gcn")
                    nc.tensor.matmul(out=gcn_ps[:], lhsT=wgcn_t[:], rhs=zT[:],
                                     start=True, stop=True)
                    gcnT = hpool.tile([H, P], f32, tag="gcnT")
                    nc.scalar.activation(out=gcnT[:], in_=gcn_ps[:], func=AF.Relu)
                    xfT_t = wpool.tile([XF, P], bf16, tag="xfT")
                    nc.sync.dma_start(out=xfT_t[:], in_=xfT[:, b * P:(b + 1) * P])
                    xlT_t = wpool.tile([XL, P], bf16, tag="xlT")
                    nc.sync.dma_start(out=xlT_t[:], in_=xlT[:, b * P:(b + 1) * P])
                    mlp_ps = psC.tile([H, P], f32, tag="mlp")
                    nc.tensor.matmul(out=mlp_ps[:], lhsT=wmlpf_t[:], rhs=xfT_t[:],
                                     start=True, stop=False)
                    nc.tensor.matmul(out=mlp_ps[:], lhsT=wmlpl_t[:], rhs=xlT_t[:],
                                     start=False, stop=True)
                    mlpT = hpool.tile([H, P], f32, tag="mlpT")
                    nc.scalar.activation(out=mlpT[:], in_=mlp_ps[:], func=AF.Relu,
                                         bias=bmlp_t[:, 0:1])
                    o_ps = psC.tile([C, P], f32, tag="o")
                    nc.tensor.matmul(out=o_ps[:], lhsT=wclsg_t[:], rhs=gcnT[:],
                                     start=True, stop=False)
                    nc.tensor.matmul(out=o_ps[:], lhsT=wclsm_t[:], rhs=mlpT[:],
                                     start=False, stop=True)
                    oT = hpool.tile([C, P], f32, tag="oT")
                    nc.scalar.activation(out=oT[:], in_=o_ps[:], func=AF.Identity,
                                         bias=bcls_t[:, 0:1])
                    # back to node-major and out
                    of_ps = psB.tile([P, C], f32, tag="tp", name="of_ps")
                    nc.tensor.transpose(out=of_ps[:], in_=oT[:],
                                        identity=ident[0:C, 0:C])
                    o_sb = hpool.tile([P, C], f32, tag="osb")
                    nc.scalar.activation(out=o_sb[:], in_=of_ps[:], func=AF.Copy)
                    nc.sync.dma_start(out=out[b * P:(b + 1) * P, :], in_=o_sb[:])
    nc.finalize()
    return nc


_CACHED = {}


def kernel(xfeat, xlabel, edge_index, W_gcn, b_gcn, W_mlp, b_mlp, W_cls, b_cls,
           _trace=False):
    import concourse.bass_utils as bass_utils

    xfeat = np.asarray(xfeat, np.float32)
    xlabel = np.asarray(xlabel, np.float32)
    edge_index = np.asarray(edge_index)
    W_gcn = np.asarray(W_gcn, np.float32)
    W_mlp = np.asarray(W_mlp, np.float32)
    b_mlp = np.asarray(b_mlp, np.float32)
    W_cls = np.asarray(W_cls, np.float32)
    b_cls = np.asarray(b_cls, np.float32)
    # b_gcn is zeros in this model; assert to be safe
    assert np.abs(np.asarray(b_gcn)).max() == 0.0

    dst = np.ascontiguousarray(edge_index[1]).astype(np.int64)
    deg = np.bincount(dst, minlength=N).astype(np.float32) + 1.0  # + self loop
    dinv = (1.0 / np.sqrt(deg)).astype(np.float32)

    cores = _preprocess(xfeat, xlabel, edge_index, dinv)

    shared = dict(
        xsbf=(dinv[:, None] * xfeat).astype(BF16),
        wgcn=W_gcn,
        wmlpf=W_mlp[:XF].astype(BF16),
        wmlpl=W_mlp[XF:].astype(BF16),
        wclsg=W_cls[:H],
        wclsm=W_cls[H:],
        bmlp=b_mlp.reshape(H, 1),
        bcls=b_cls.reshape(C, 1),
    )
    in_maps = [{**shared, **c} for c in cores]

    if "nc" not in _CACHED:
        _CACHED["nc"] = _build_bass()
    nc = _CACHED["nc"]

    res = bass_utils.run_bass_kernel_spmd(
        nc, in_maps, core_ids=list(range(NCORES)), trace=_trace,
    )
    out = np.concatenate(
        [res.results[c]["out"][:NSHARD] for c in range(NCORES)], axis=0
    )
    if _trace:
        kernel._last_exec_time_ns = res.exec_time_ns
        kernel._last_results = res
    return out


# revision 6
# speedup vs baseline: 2.2102x; 2.2102x over previous
"""GCN + MLP concat kernel for Trainium2, 8-core SPMD.

Model (reference.py):
    gcn_out = relu(gcn_conv(xfeat, edge_index, W_gcn, b_gcn))      # symmetric-norm GCN
    mlp_out = relu(concat(xfeat, xlabel) @ W_mlp + b_mlp)
    out     = concat(gcn_out, mlp_out) @ W_cls + b_cls

Shapes: N=100000 nodes, E=1600000 edges, XF=128, XL=40, H=128, C=40.

Strategy: shard dst nodes across 8 cores (12500 each, padded to 12800 =
100 blocks of 128); each core gathers its incoming edges' source rows;
weights replicated.

v3 design:
  - norm factorization: gather xs = dinv (.) xfeat (bf16); selection
    matrices are 0/1 (streamed fp8, exact); dinv[dst] applied via DVE
    multiply with a broadcast dinv row at PSUM evacuation.
  - feature-major strips: aggregation accumulates zT [128f, 512dst]
    (4 blocks) in ONE psum bank via matmul(lhsT=G_tile, rhs=S_tile),
    so PE runs 68 matmuls back-to-back per strip with no interleaved
    ACT dependencies (v2's per-block chain serialized the machine).
  - 512-slot cells (4 tiles per (block, quartile)); the ~1.8% of edges
    overflowing a cell are folded host-side into the self-loop tensor
    xss (z += I-matmul over xss rows covers self-loop + spill).
  - head in 512-wide matmuls; node-major cls output (no transposes);
    bias via DVE add with a broadcast tile; batched per-strip streams.
"""

import numpy as np
import ml_dtypes

N, E = 100000, 1600000
XF, XL, H, C = 128, 40, 128, 40
NCORES = 8
NSHARD = N // NCORES          # 12500 dst nodes per core
P = 128
NBLK = 100                    # dst blocks per core (12800 padded rows)
NPAD = NBLK * P               # 12800
NQ = 4                        # src-table quartiles (int16 index range)
QROWS = N // NQ               # 25000
TBQ = 4                       # gather tiles per (block, quartile) - 512 slots
STRIP = 4                     # blocks per strip / gather group
NSTRIP = NBLK // STRIP        # 25
TSB = NQ * STRIP * TBQ        # 64 tiles per strip
TTOT = NBLK * NQ * TBQ        # 1600 tiles per core
CALL = STRIP * TBQ * P        # 2048 slots per gather call (per quartile)
SW = STRIP * P                # 512 dst columns per strip

BF16 = ml_dtypes.bfloat16
FP8 = ml_dtypes.float8_e4m3fn


def _preprocess(xfeat, xlabel, edge_index, dinv):
    """Host-side sharding/layout. Returns per-core input dicts' arrays."""
    src = np.ascontiguousarray(edge_index[0]).astype(np.int64)
    dst = np.ascontiguousarray(edge_index[1]).astype(np.int64)

    core = dst // NSHARD
    blk = (dst % NSHARD) // P
    qrt = src // QROWS
    dloc = (dst % NSHARD) % P  # position within block

    # order edges by (core, block, quartile, src)
    order = np.lexsort((src, qrt, blk, core))
    src_s = src[order]
    dst_s = dst[order]
    core_s = core[order]
    blk_s = blk[order]
    qrt_s = qrt[order]
    dloc_s = dloc[order]

    cell = ((core_s * NBLK + blk_s) * NQ + qrt_s)  # global (c,b,q) cell id
    ncells = NCORES * NBLK * NQ
    counts = np.bincount(cell, minlength=ncells)
    cell_starts = np.zeros(ncells, np.int64)
    cell_starts[1:] = np.cumsum(counts)[:-1]
    within = np.arange(len(src_s)) - cell_starts[cell]

    CAP = TBQ * P  # 512 on-device slots per cell
    on_dev = within < CAP

    # spill edges (cell overflow) are folded host-side into xss
    sp_src = src_s[~on_dev]
    sp_dst = dst_s[~on_dev]

    # global slot id per on-device edge; slot layout per core:
    # for g in NSTRIP: for q in NQ: for b in STRIP: TBQ tiles of 128 slots
    b_, q_ = blk_s[on_dev], qrt_s[on_dev]
    tile_base = ((b_ // STRIP) * TSB + q_ * (STRIP * TBQ)
                 + (b_ % STRIP) * TBQ)
    slot = tile_base * P + within[on_dev]
    gslot = core_s[on_dev] * (TTOT * P) + slot

    total_slots = NCORES * TTOT * P
    idx_flat = np.zeros(total_slots, np.int16)
    dloc_flat = np.zeros(total_slots, np.int64)
    val_flat = np.zeros(total_slots, np.float32)
    idx_flat[gslot] = (src_s[on_dev] - q_ * QROWS).astype(np.int16)
    dloc_flat[gslot] = dloc_s[on_dev]
    val_flat[gslot] = 1.0

    # host spill aggregate in fp32: spill_sum[d] = sum xs[src]
    xs32 = dinv[:, None] * xfeat                      # [N, XF] fp32
    spill = np.zeros((N, XF), np.float32)
    np.add.at(spill, sp_dst, xs32[sp_src])

    cores = []
    for c in range(NCORES):
        s0, s1 = c * TTOT * P, (c + 1) * TTOT * P
        idx_c = idx_flat[s0:s1]
        # idx wrap for dma_gather: per call region, idx j at [j%16, j//16],
        # replicated to the 8 16-partition groups.
        ncalls = TTOT * P // CALL
        w = idx_c.reshape(ncalls, CALL // 16, 16)          # [call, col, 16]
        w = np.transpose(w, (2, 0, 1)).reshape(16, TTOT * P // 16)
        idx_wrapped = np.tile(w, (8, 1))

        # host-built 0/1 selection tiles S^T (fp8): [128 slots, TTOT, 128 dst]
        sarr = np.zeros((P, TTOT, P), FP8)
        pp = (np.arange(TTOT * P) % P)
        tt = (np.arange(TTOT * P) // P)
        sarr[pp, tt, dloc_flat[s0:s1]] = val_flat[s0:s1].astype(FP8)
        sarr = sarr.reshape(P, TTOT * P)

        nodes0 = c * NSHARD
        xf_shard = np.zeros((NPAD, XF), np.float32)
        xf_shard[:NSHARD] = xfeat[nodes0:nodes0 + NSHARD]
        xl_shard = np.zeros((NPAD, XL), np.float32)
        xl_shard[:NSHARD] = xlabel[nodes0:nodes0 + NSHARD]
        d_pad = np.zeros(NPAD, np.float32)
        d_pad[:NSHARD] = dinv[nodes0:nodes0 + NSHARD]
        # self-loop + host-folded spill rows for the identity matmul
        xss = (xs32[nodes0:nodes0 + NSHARD] + spill[nodes0:nodes0 + NSHARD])
        xss = np.concatenate([xss, np.zeros((NPAD - NSHARD, XF), np.float32)])

        cores.append(dict(
            idx=idx_wrapped, sarr=sarr,
            xss=xss.astype(BF16),
            xfT=np.ascontiguousarray(xf_shard.T).astype(BF16),
            xlT=np.ascontiguousarray(xl_shard.T).astype(BF16),
            dinvT=np.ascontiguousarray(
                np.broadcast_to(d_pad[None, :], (P, NPAD))),
        ))
    return cores


def _build_bass():
    import concourse.mybir as mybir
    import concourse.tile as tile
    from concourse import bacc
    from concourse.masks import make_identity

    f32 = mybir.dt.float32
    bf16 = mybir.dt.bfloat16
    f8 = mybir.dt.float8e4
    i16 = mybir.dt.int16
    AF = mybir.ActivationFunctionType
    ALU = mybir.AluOpType

    nc = bacc.Bacc(None, target_bir_lowering=False, num_swdge_queues=4)

    xsbf = nc.dram_tensor("xsbf", [N, XF], bf16, kind="ExternalInput")
    idx = nc.dram_tensor("idx", [P, TTOT * P // 16], i16, kind="ExternalInput")
    sarr = nc.dram_tensor("sarr", [P, TTOT * P], f8, kind="ExternalInput")
    xss = nc.dram_tensor("xss", [NPAD, XF], bf16, kind="ExternalInput")
    xfT = nc.dram_tensor("xfT", [XF, NPAD], bf16, kind="ExternalInput")
    xlT = nc.dram_tensor("xlT", [XL, NPAD], bf16, kind="ExternalInput")
    dinvT = nc.dram_tensor("dinvT", [P, NPAD], f32, kind="ExternalInput")
    wgcn = nc.dram_tensor("wgcn", [XF, H], bf16, kind="ExternalInput")
    wmlpf = nc.dram_tensor("wmlpf", [XF, H], bf16, kind="ExternalInput")
    wmlpl = nc.dram_tensor("wmlpl", [XL, H], bf16, kind="ExternalInput")
    wclsg = nc.dram_tensor("wclsg", [H, C], f32, kind="ExternalInput")
    wclsm = nc.dram_tensor("wclsm", [H, C], f32, kind="ExternalInput")
    bmlp = nc.dram_tensor("bmlp", [H, 1], f32, kind="ExternalInput")
    bclsb = nc.dram_tensor("bclsb", [P, STRIP * C], f32, kind="ExternalInput")

    out = nc.dram_tensor("out", [NPAD, C], f32, kind="ExternalOutput")

    ICALL = CALL * NQ // 16  # idx columns per strip (512)

    with tile.TileContext(nc) as tc:
        with (
            tc.tile_pool(name="const", bufs=1) as cpool,
            tc.tile_pool(name="meta", bufs=3) as mpool,
            tc.tile_pool(name="gbuf", bufs=3) as gpool,
            tc.tile_pool(name="sbufS", bufs=3) as spool,
            tc.tile_pool(name="work", bufs=3) as wpool,
            tc.tile_pool(name="selfp", bufs=8) as fpool,
            tc.tile_pool(name="head", bufs=2) as hpool,
            tc.tile_pool(name="psZ", bufs=2, space="PSUM") as psZ,
            tc.tile_pool(name="psG", bufs=2, space="PSUM") as psG,
            tc.tile_pool(name="psM", bufs=2, space="PSUM") as psM,
            tc.tile_pool(name="psO", bufs=2, space="PSUM") as psO,
        ):
            ident_bf = cpool.tile([P, P], bf16)
            make_identity(nc, ident_bf[:])
            wgcn_t = cpool.tile([XF, H], bf16)
            nc.sync.dma_start(out=wgcn_t[:], in_=wgcn[:, :])
            wmlpf_t = cpool.tile([XF, H], bf16)
            nc.sync.dma_start(out=wmlpf_t[:], in_=wmlpf[:, :])
            wmlpl_t = cpool.tile([XL, H], bf16)
            nc.sync.dma_start(out=wmlpl_t[:], in_=wmlpl[:, :])
            wclsg_t = cpool.tile([H, C], f32)
            nc.sync.dma_start(out=wclsg_t[:], in_=wclsg[:, :])
            wclsm_t = cpool.tile([H, C], f32)
            nc.sync.dma_start(out=wclsm_t[:], in_=wclsm[:, :])
            bmlp_t = cpool.tile([H, 1], f32)
            nc.sync.dma_start(out=bmlp_t[:], in_=bmlp[:, :])
            bclsb_t = cpool.tile([P, STRIP * C], f32)
            nc.sync.dma_start(out=bclsb_t[:], in_=bclsb[:, :])

            for st in range(NSTRIP):
                c0 = st * SW  # first dst column of strip
                idx_t = mpool.tile([P, ICALL], i16, tag="idx")
                nc.sync.dma_start(
                    out=idx_t[:], in_=idx[:, st * ICALL:(st + 1) * ICALL])
                g_t = gpool.tile([P, TSB, P], bf16, tag="g")
                for q in range(NQ):
                    nc.gpsimd.dma_gather(
                        g_t[:, q * STRIP * TBQ:(q + 1) * STRIP * TBQ, :],
                        xsbf[q * QROWS:(q + 1) * QROWS, :],
                        idx_t[:, q * (CALL // 16):(q + 1) * (CALL // 16)],
                        CALL, CALL, P,
                        single_packet=False,
                        queue_num=(st * NQ + q) % 4,
                    )
                s_t = spool.tile([P, TSB * P], f8, tag="sm")
                nc.sync.dma_start(
                    out=s_t[:], in_=sarr[:, st * TSB * P:(st + 1) * TSB * P])
                dv_t = wpool.tile([P, SW], f32, tag="dv")
                nc.scalar.dma_start(out=dv_t[:], in_=dinvT[:, c0:c0 + SW])
                xfT_t = wpool.tile([XF, SW], bf16, tag="xfT")
                nc.sync.dma_start(out=xfT_t[:], in_=xfT[:, c0:c0 + SW])
                xlT_t = wpool.tile([XL, SW], bf16, tag="xlT")
                nc.sync.dma_start(out=xlT_t[:], in_=xlT[:, c0:c0 + SW])

                # aggregation: zT[f, d] accumulated strip-wide in one bank
                zT_ps = psZ.tile([P, SW], f32, tag="z")
                for bl in range(STRIP):
                    zcol = zT_ps[:, bl * P:(bl + 1) * P]
                    for q in range(NQ):
                        for k in range(TBQ):
                            t = q * (STRIP * TBQ) + bl * TBQ + k
                            nc.tensor.matmul(
                                out=zcol,
                                lhsT=g_t[:, t, :],
                                rhs=s_t[:, t * P:(t + 1) * P],
                                start=(q == 0 and k == 0),
                                stop=False,
                            )
                    # self-loop + host-folded spill rows
                    xss_t = fpool.tile([P, XF], bf16, tag="xss")
                    nc.scalar.dma_start(
                        out=xss_t[:],
                        in_=xss[(st * STRIP + bl) * P:(st * STRIP + bl + 1) * P, :])
                    nc.tensor.matmul(
                        out=zcol, lhsT=xss_t[:], rhs=ident_bf[:],
                        start=False, stop=True,
                    )
                # PSUM evacuation fused with exact dinv[dst] scaling (DVE)
                zT_sb = wpool.tile([P, SW], bf16, tag="zsb")
                nc.vector.tensor_tensor(
                    out=zT_sb[:], in0=zT_ps[:], in1=dv_t[:], op=ALU.mult)
                # heads (feature-major, 512-wide)
                gcn_ps = psG.tile([H, SW], f32, tag="gcn")
                nc.tensor.matmul(out=gcn_ps[:], lhsT=wgcn_t[:], rhs=zT_sb[:],
                                 start=True, stop=True)
                gcnT = hpool.tile([H, SW], f32, tag="gcnT")
                nc.scalar.activation(out=gcnT[:], in_=gcn_ps[:], func=AF.Relu)
                mlp_ps = psM.tile([H, SW], f32, tag="mlp")
                nc.tensor.matmul(out=mlp_ps[:], lhsT=wmlpf_t[:], rhs=xfT_t[:],
                                 start=True, stop=False)
                nc.tensor.matmul(out=mlp_ps[:], lhsT=wmlpl_t[:], rhs=xlT_t[:],
                                 start=False, stop=True)
                mlpT = hpool.tile([H, SW], f32, tag="mlpT")
                nc.scalar.activation(out=mlpT[:], in_=mlp_ps[:], func=AF.Relu,
                                     bias=bmlp_t[:, 0:1])
                # classifier, node-major per block (no output transposes)
                o_ps = psO.tile([P, STRIP * C], f32, tag="o")
                for bl in range(STRIP):
                    nc.tensor.matmul(
                        out=o_ps[:, bl * C:(bl + 1) * C],
                        lhsT=gcnT[:, bl * P:(bl + 1) * P], rhs=wclsg_t[:],
                        start=True, stop=False)
                    nc.tensor.matmul(
                        out=o_ps[:, bl * C:(bl + 1) * C],
                        lhsT=mlpT[:, bl * P:(bl + 1) * P], rhs=wclsm_t[:],
                        start=False, stop=True)
                o_sb = hpool.tile([P, STRIP * C], f32, tag="osb")
                nc.vector.tensor_tensor(
                    out=o_sb[:], in0=o_ps[:], in1=bclsb_t[:], op=ALU.add)
                for bl in range(STRIP):
                    b = st * STRIP + bl
                    nc.sync.dma_start(
                        out=out[b * P:(b + 1) * P, :],
                        in_=o_sb[:, bl * C:(bl + 1) * C])
    nc.finalize()
    return nc


_CACHED = {}


def kernel(xfeat, xlabel, edge_index, W_gcn, b_gcn, W_mlp, b_mlp, W_cls, b_cls,
           _trace=False):
    import concourse.bass_utils as bass_utils

    xfeat = np.asarray(xfeat, np.float32)
    xlabel = np.asarray(xlabel, np.float32)
    edge_index = np.asarray(edge_index)
    W_gcn = np.asarray(W_gcn, np.float32)
    W_mlp = np.asarray(W_mlp, np.float32)
    b_mlp = np.asarray(b_mlp, np.float32)
    W_cls = np.asarray(W_cls, np.float32)
    b_cls = np.asarray(b_cls, np.float32)
    # b_gcn is zeros in this model; assert to be safe
    assert np.abs(np.asarray(b_gcn)).max() == 0.0

    dst = np.ascontiguousarray(edge_index[1]).astype(np.int64)
    deg = np.bincount(dst, minlength=N).astype(np.float32) + 1.0  # + self loop
    dinv = (1.0 / np.sqrt(deg)).astype(np.float32)

    cores = _preprocess(xfeat, xlabel, edge_index, dinv)

    shared = dict(
        xsbf=(dinv[:, None] * xfeat).astype(BF16),
        wgcn=W_gcn.astype(BF16),
        wmlpf=W_mlp[:XF].astype(BF16),
        wmlpl=W_mlp[XF:].astype(BF16),
        wclsg=W_cls[:H],
        wclsm=W_cls[H:],
        bmlp=b_mlp.reshape(H, 1),
        bclsb=np.ascontiguousarray(
            np.broadcast_to(np.tile(b_cls, STRIP)[None, :], (P, STRIP * C))),
    )
    in_maps = [{**shared, **c} for c in cores]

    if "nc" not in _CACHED:
        _CACHED["nc"] = _build_bass()
    nc = _CACHED["nc"]

    res = bass_utils.run_bass_kernel_spmd(
        nc, in_maps, core_ids=list(range(NCORES)), trace=_trace,
    )
    out = np.concatenate(
        [res.results[c]["out"][:NSHARD] for c in range(NCORES)], axis=0
    )
    if _trace:
        kernel._last_exec_time_ns = res.exec_time_ns
        kernel._last_results = res
    return out
